# revision 17
# baseline (speedup 1.0000x reference)
"""CenterPointHead Bass/Trainium2 kernel.

Reference computation (all convs stride 1):
  shared  = relu(bn(conv3x3(bev[4,256,300,300], w_shared[64,256,3,3])))
  hm      = relu(bn(conv3x3(shared, w_hm1[64,64,3,3])))
  heatmap = conv1x1(hm, w_hm2[3,64]) + b_hm2
  rg      = relu(bn(conv3x3(shared, w_reg1[64,64,3,3])))
  box_reg = conv1x1(rg, w_reg2[8,64]) + b_reg2
  returns (heatmap[4,3,300,300], box_reg[4,8,300,300])

Sharding: 8 cores = batch(4) x H-halves(2). Each core computes 150 output
rows of one image, with input halo rows (zero-filled outside the image)
so the single SPMD program is uniform across cores.

Per-core dataflow (all BN scales folded into conv weights host-side):
  conv1: shared rows computed in pairs as M=128 fp32r matmuls; lhsT columns
         0:64 carry the even row's tap, 64:128 the odd row's tap shifted by
         one input row (zero blocks at the 4-row chain boundaries).
         KC(2) x kx(3) x 4 input rows = 24 matmuls (N=300) per row pair.
  conv2+conv3 merged: output channels of hm1|reg1 stacked -> M=128.
         Shared rows stored pairwise in [128, WP] tiles (row 2t at
         partitions 0:64, row 2t+1 at 64:128) so one K=128 matmul covers two
         vertical taps; the leftover tap is a K=64 matmul at src partition
         base 0/64. 6 matmuls per output row.
  heads: w_hm2|w_reg2 block-diagonal -> one K=128, M=11 matmul per row.
"""

import numpy as np

import concourse.bacc as bacc
import concourse.mybir as mybir
import concourse.tile as tile
from concourse.bass_utils import run_bass_kernel_spmd

F32 = mybir.dt.float32
F32R = mybir.dt.float32r
RELU = mybir.ActivationFunctionType.Relu

B, C, H, W = 4, 256, 300, 300
HC = 64          # hidden channels
KC = 2           # input channel chunks of 128
NOUT = 11        # 3 heatmap + 8 box_reg channels
HALF = 150       # output rows per core
NSH = HALF + 2   # shared rows computed per core (1 halo row each side)
NIN = HALF + 4   # input rows per core (2 halo rows each side)
WP = W + 2       # width padded with one zero column each side
G = 10           # output rows batched per store DMA
EPS = 1e-5

_CACHED_NC = None


def _build_nc(n_pairs=NSH // 2):
    nc = bacc.Bacc()

    x_d = nc.dram_tensor("x", [KC, NIN, 128, WP], F32R, kind="ExternalInput")
    w1_d = nc.dram_tensor("w1", [128, 24 * 128], F32R, kind="ExternalInput")
    w23p_d = nc.dram_tensor("w23p", [128, 6 * 128], F32R, kind="ExternalInput")
    w23_d = nc.dram_tensor("w23", [128, 9 * 128], F32R, kind="ExternalInput")
    wh_d = nc.dram_tensor("wh", [128, 16], F32R, kind="ExternalInput")
    cb_d = nc.dram_tensor("cb", [128, 8], F32, kind="ExternalInput")
    out_d = nc.dram_tensor("out", [HALF, NOUT, W], F32, kind="ExternalOutput")

    with tile.TileContext(nc) as tc:
        with (
            tc.tile_pool(name="pw", bufs=1) as pw,
            tc.tile_pool(name="px", bufs=16) as px,
            tc.tile_pool(name="psh", bufs=7) as psh,
            tc.tile_pool(name="phm", bufs=4) as phm,
            tc.tile_pool(name="pob", bufs=2) as pob,
            tc.tile_pool(name="pp1", bufs=2, space="PSUM") as pp1,
            tc.tile_pool(name="pp23", bufs=4, space="PSUM") as pp23,
            tc.tile_pool(name="pph", bufs=2, space="PSUM") as pph,
        ):
            xtiles = {}    # (input_row, chunk) -> AP
            shtiles = {}   # pair index -> AP ([128, WP]; row 2k at [0:64], 2k+1 at [64:128])
            ob = None

            def load_x(m):
                for c in range(KC):
                    xt = px.tile([128, WP], F32R, tag="xt", name=f"xt_{m}_{c}")
                    nc.sync.dma_start(out=xt, in_=x_d[c, m])
                    xtiles[(m, c)] = xt

            def conv1_pair(k):
                ps = pp1.tile([128, W], F32, tag="ps1", name=f"ps1_{k}")
                nmm = KC * 3 * 4
                i = 0
                for c in range(KC):
                    for kx in range(3):
                        for o in range(4):
                            xt = xtiles[(2 * k + o, c)]
                            lhs = w1_sb[:, ((c * 3 + kx) * 4 + o) * 128:
                                        ((c * 3 + kx) * 4 + o + 1) * 128]
                            nc.tensor.matmul(ps, lhs, xt[:, kx:kx + W],
                                             start=(i == 0), stop=(i == nmm - 1))
                            i += 1
                sh = psh.tile([128, WP], F32R, tag="sh", name=f"sh_{k}")
                nc.vector.memset(sh[:, 0:1].bitcast(mybir.dt.uint32), 0)
                nc.vector.memset(sh[:, WP - 1:WP].bitcast(mybir.dt.uint32), 0)
                for jj in range(2):
                    j = 2 * k + jj
                    sl = slice(64 * jj, 64 * jj + HC)
                    if j == 0:           # top halo row: masked per core
                        bias, scale = cb_sb[sl, 1:2], cb_sb[sl, 3:4]
                    elif j == NSH - 1:   # bottom halo row: masked per core
                        bias, scale = cb_sb[sl, 2:3], cb_sb[sl, 4:5]
                    else:
                        bias, scale = cb_sb[sl, 0:1], 1.0
                    nc.scalar.activation(sh[sl, 1:1 + W], ps[sl, :], RELU,
                                         bias=bias, scale=scale)
                shtiles[k] = sh

            def conv23_quad(q0, nq=2):
                # outputs y = 2q0 .. 2q0+3 (two row pairs q0, q0+1)
                qs = [q0 + i for i in range(nq)]
                ps = {}
                for q in qs:
                    for yi in range(2):
                        ps[2 * q + yi] = pp23.tile([128, W], F32, tag="ps23",
                                                   name=f"ps23_{q}_{yi}")
                # all K=128 paired-tap matmuls (A) first
                for q in qs:
                    for kx in range(3):
                        for yi in range(2):
                            t = q + yi
                            lhs = w23p_sb[:, (yi * 3 + kx) * 128:
                                          (yi * 3 + kx + 1) * 128]
                            nc.tensor.matmul(ps[2 * q + yi], lhs,
                                             shtiles[t][:, kx:kx + W],
                                             start=(kx == 0), stop=False)
                # then all K=64 leftover taps as row-group pairs (B)
                for q in qs:
                    for kx in range(3):
                        for yi in range(2):
                            t = q + 1 - yi
                            p = yi
                            tap = (1 - yi) * 2 * 3 + kx
                            lhs = w23_sb[64 * p:64 * p + 64,
                                         tap * 128:(tap + 1) * 128]
                            rhs = shtiles[t][64 * p:64 * p + 64, kx:kx + W]
                            nc.tensor.matmul(ps[2 * q + yi], lhs, rhs,
                                             start=False, stop=(kx == 2))
                hm = {}
                for q in qs:
                    for yi in range(2):
                        h = phm.tile([128, W], F32R, tag="hm",
                                     name=f"hm_{q}_{yi}")
                        nc.scalar.activation(h, ps[2 * q + yi], RELU,
                                             bias=cb_sb[:, 5:6])
                        hm[2 * q + yi] = h
                return hm

            def heads_quad(q0, hm, nq=2):
                nonlocal ob
                for y in range(2 * q0, 2 * q0 + 2 * nq):
                    if ob is None:
                        ob = pob.tile([NOUT, G, W], F32, tag="ob",
                                      name=f"ob_{y}")
                    ph = pph.tile([128, W], F32, tag="psh", name=f"psh_{y}")
                    nc.tensor.matmul(ph[0:NOUT, :], wh_sb[:, 0:NOUT],
                                     hm[y][:, 0:W], start=True, stop=True)
                    nc.vector.tensor_scalar_add(
                        ob[:, y % G, :], ph[0:NOUT, :], cb_sb[0:NOUT, 6:7])
                    if y % G == G - 1:
                        y0 = y - G + 1
                        nc.sync.dma_start(
                            out=out_d[y0:y0 + G].rearrange("g c w -> c g w"),
                            in_=ob)
                        ob = None

            load_x(0), load_x(1)
            # conv1 weights right after the first x rows; the rest are not
            # needed until iteration 2+, so they go behind the prefetch
            w1_sb = pw.tile([128, 24 * 128], F32R)
            nc.sync.dma_start(out=w1_sb, in_=w1_d[:, :])
            load_x(2), load_x(3)
            cb_sb = pw.tile([128, 8], F32)
            nc.sync.dma_start(out=cb_sb, in_=cb_d[:, :])
            w23p_sb = pw.tile([128, 6 * 128], F32R)
            nc.sync.dma_start(out=w23p_sb, in_=w23p_d[:, :])
            w23_sb = pw.tile([128, 9 * 128], F32R)
            nc.sync.dma_start(out=w23_sb, in_=w23_d[:, :])
            wh_sb = pw.tile([128, 16], F32R)
            nc.sync.dma_start(out=wh_sb, in_=wh_d[:, :])
            n_q = n_pairs - 1          # output row pairs
            NQ0_MAX = ((n_q - 1) // 2) * 2  # last quad start (may be 1-pair)
            NQ_LAST = n_q - NQ0_MAX
            for k in range(NQ0_MAX + 4):
                if k < n_pairs:
                    if k > 0:
                        load_x(2 * k + 2), load_x(2 * k + 3)
                    conv1_pair(k)
                    # drop input tiles no longer needed
                    for c in range(KC):
                        xtiles.pop((2 * k - 2, c), None)
                        xtiles.pop((2 * k - 1, c), None)
                if k >= 3 and k % 2 == 1 and k - 3 <= NQ0_MAX:
                    q0 = k - 3
                    nq = 2 if q0 < NQ0_MAX else NQ_LAST
                    hm = conv23_quad(q0, nq)
                    heads_quad(q0, hm, nq)
                    shtiles.pop(q0 - 1, None)

    nc.compile()
    return nc


def _prep_shards(inputs):
    f = np.float32
    bev = np.ascontiguousarray(inputs["bev_features"], dtype=f)

    def bnfold(w, g, b, m, v):
        s = (g / np.sqrt(v + EPS)).astype(f)
        return (w * s[:, None, None, None]).astype(f), (b - m * s).astype(f)

    w1f, b1 = bnfold(inputs["w_shared"], inputs["g_shared"], inputs["b_shared"],
                     inputs["m_shared"], inputs["v_shared"])
    whm, bhm = bnfold(inputs["w_hm1"], inputs["g_hm1"], inputs["b_hm1"],
                      inputs["m_hm1"], inputs["v_hm1"])
    wrg, brg = bnfold(inputs["w_reg1"], inputs["g_reg1"], inputs["b_reg1"],
                      inputs["m_reg1"], inputs["v_reg1"])

    # conv1 row-pair weights: [128 k, ((c*3+kx)*4 + o) * 128 + m]
    # column block m<64 = tap ky=o for the even row (zero when o==3),
    # block m>=64 = tap ky=o-1 for the odd row (zero when o==0)
    w1_np = np.zeros((128, 24, 128), f)
    for c in range(KC):
        for kx in range(3):
            for o in range(4):
                col = (c * 3 + kx) * 4 + o
                if o <= 2:
                    w1_np[:, col, :HC] = w1f[:, c * 128:(c + 1) * 128, o, kx].T
                if o >= 1:
                    w1_np[:, col, HC:] = w1f[:, c * 128:(c + 1) * 128, o - 1, kx].T
    w1_np = np.ascontiguousarray(w1_np.reshape(128, 24 * 128))

    # conv2+3 merged per-tap weights [64 k, tap, m] (m<64 hm1 | m>=64 reg1)
    w23_half = np.zeros((64, 9, 128), f)
    for t in range(9):
        ky, kx = divmod(t, 3)
        w23_half[:, t, :HC] = whm[:, :, ky, kx].T
        w23_half[:, t, HC:] = wrg[:, :, ky, kx].T
    # duplicated across partition halves (for K=64 matmuls at src base 0/64)
    w23_np = np.ascontiguousarray(
        np.tile(w23_half, (2, 1, 1)).reshape(128, 9 * 128))
    # K-packed vertical tap pairs: [p, par*3+kx, m] with p<64 -> tap ky=par,
    # p>=64 -> tap ky=par+1 (par = output-row parity)
    w23p_np = np.zeros((128, 6, 128), f)
    for par in range(2):
        for kx in range(3):
            w23p_np[:64, par * 3 + kx] = w23_half[:, par * 3 + kx]
            w23p_np[64:, par * 3 + kx] = w23_half[:, (par + 1) * 3 + kx]
    w23p_np = np.ascontiguousarray(w23p_np.reshape(128, 6 * 128))

    # head weights, block diagonal: [128 k, 11]
    wh_np = np.zeros((128, 16), f)
    wh_np[:HC, 0:3] = inputs["w_hm2"][:, :, 0, 0].T
    wh_np[HC:, 3:NOUT] = inputs["w_reg2"][:, :, 0, 0].T

    bh_np = np.concatenate([inputs["b_hm2"], inputs["b_reg2"]]).astype(f)

    in_maps = []
    for core in range(8):
        b, half = divmod(core, 2)
        r0 = half * HALF - 2
        x_np = np.zeros((KC, NIN, 128, WP), f)
        lo, hi = max(0, r0), min(H, r0 + NIN)
        src = bev[b, :, lo:hi, :].reshape(KC, 128, hi - lo, W)
        x_np[:, lo - r0:hi - r0, :, 1:1 + W] = src.transpose(0, 2, 1, 3)

        m_top = f(0.0 if half == 0 else 1.0)
        m_bot = f(0.0 if half == 1 else 1.0)
        cb_np = np.zeros((128, 8), f)
        b1d = np.tile(b1, 2)
        cb_np[:, 0] = b1d
        cb_np[:, 1] = b1d * m_top
        cb_np[:, 2] = b1d * m_bot
        cb_np[:, 3] = m_top
        cb_np[:, 4] = m_bot
        cb_np[:, 5] = np.concatenate([bhm, brg])
        cb_np[0:NOUT, 6] = bh_np
        cb_np[32:32 + NOUT, 6] = bh_np

        in_maps.append({"x": x_np, "w1": w1_np, "w23": w23_np,
                        "w23p": w23p_np, "wh": wh_np, "cb": cb_np})
    return in_maps


def _run(inputs, trace=False):
    global _CACHED_NC
    if _CACHED_NC is None:
        _CACHED_NC = _build_nc()
    nc = _CACHED_NC
    in_maps = _prep_shards(inputs)
    kw = {}
    if trace:
        # dev-only profiling path; requires the sibling ntff_shim module
        import os
        import shutil
        import ntff_shim  # noqa: F401
        shutil.rmtree("/tmp/ntff_kernel", ignore_errors=True)
        os.makedirs("/tmp/ntff_kernel", exist_ok=True)
        kw = dict(tmpdir="/tmp/ntff_kernel", trace_kwargs={})
    res = run_bass_kernel_spmd(nc, in_maps, list(range(8)), trace=trace, **kw)

    heatmap = np.empty((B, 3, H, W), np.float32)
    box_reg = np.empty((B, 8, H, W), np.float32)
    for core in range(8):
        b, half = divmod(core, 2)
        o = res.results[core]["out"]  # [150, 11, 300]
        rows = slice(half * HALF, half * HALF + HALF)
        heatmap[b, :, rows, :] = o[:, 0:3, :].transpose(1, 0, 2)
        box_reg[b, :, rows, :] = o[:, 3:NOUT, :].transpose(1, 0, 2)
    return (heatmap, box_reg), res


def kernel(**inputs):
    out, _ = _run(inputs)
    return out


# revision 18
# speedup vs baseline: 1.0038x; 1.0038x over previous
"""CenterPointHead Bass/Trainium2 kernel.

Reference computation (all convs stride 1):
  shared  = relu(bn(conv3x3(bev[4,256,300,300], w_shared[64,256,3,3])))
  hm      = relu(bn(conv3x3(shared, w_hm1[64,64,3,3])))
  heatmap = conv1x1(hm, w_hm2[3,64]) + b_hm2
  rg      = relu(bn(conv3x3(shared, w_reg1[64,64,3,3])))
  box_reg = conv1x1(rg, w_reg2[8,64]) + b_reg2
  returns (heatmap[4,3,300,300], box_reg[4,8,300,300])

Sharding: 8 cores = batch(4) x H-halves(2). Each core computes 150 output
rows of one image, with input halo rows (zero-filled outside the image)
so the single SPMD program is uniform across cores.

Per-core dataflow (all BN scales folded into conv weights host-side):
  conv1: shared rows computed in pairs as M=128 fp32r matmuls; lhsT columns
         0:64 carry the even row's tap, 64:128 the odd row's tap shifted by
         one input row (zero blocks at the 4-row chain boundaries).
         KC(2) x kx(3) x 4 input rows = 24 matmuls (N=300) per row pair.
  conv2+conv3 merged: output channels of hm1|reg1 stacked -> M=128.
         Shared rows stored pairwise in [128, WP] tiles (row 2t at
         partitions 0:64, row 2t+1 at 64:128) so one K=128 matmul covers two
         vertical taps; the leftover tap is a K=64 matmul at src partition
         base 0/64. 6 matmuls per output row.
  heads: w_hm2|w_reg2 block-diagonal -> one K=128, M=11 matmul per row.
"""

import numpy as np

import concourse.bacc as bacc
import concourse.mybir as mybir
import concourse.tile as tile
from concourse.bass_utils import run_bass_kernel_spmd

F32 = mybir.dt.float32
F32R = mybir.dt.float32r
RELU = mybir.ActivationFunctionType.Relu

B, C, H, W = 4, 256, 300, 300
HC = 64          # hidden channels
KC = 2           # input channel chunks of 128
NOUT = 11        # 3 heatmap + 8 box_reg channels
HALF = 150       # output rows per core
NSH = HALF + 2   # shared rows computed per core (1 halo row each side)
NIN = HALF + 4   # input rows per core (2 halo rows each side)
WP = W + 2       # width padded with one zero column each side
G = 10           # output rows batched per store DMA
EPS = 1e-5

_CACHED_NC = None


def _build_nc(n_pairs=NSH // 2):
    nc = bacc.Bacc()

    x_d = nc.dram_tensor("x", [KC, NIN, 128, WP], F32R, kind="ExternalInput")
    w1_d = nc.dram_tensor("w1", [128, 24 * 128], F32R, kind="ExternalInput")
    w23p_d = nc.dram_tensor("w23p", [128, 6 * 128], F32R, kind="ExternalInput")
    w23_d = nc.dram_tensor("w23", [128, 9 * 128], F32R, kind="ExternalInput")
    wh_d = nc.dram_tensor("wh", [128, 16], F32R, kind="ExternalInput")
    cb_d = nc.dram_tensor("cb", [128, 8], F32, kind="ExternalInput")
    out_d = nc.dram_tensor("out", [HALF, NOUT, W], F32, kind="ExternalOutput")

    with tile.TileContext(nc) as tc:
        with (
            tc.tile_pool(name="pw", bufs=1) as pw,
            tc.tile_pool(name="px", bufs=16) as px,
            tc.tile_pool(name="psh", bufs=7) as psh,
            tc.tile_pool(name="phm", bufs=4) as phm,
            tc.tile_pool(name="pob", bufs=3) as pob,
            tc.tile_pool(name="pp1", bufs=4, space="PSUM") as pp1,
            tc.tile_pool(name="pp23", bufs=4, space="PSUM") as pp23,
        ):
            xtiles = {}    # (input_row, chunk) -> AP
            shtiles = {}   # pair index -> AP ([128, WP]; row 2k at [0:64], 2k+1 at [64:128])
            ob = None

            def load_x(m, chunks=range(KC)):
                for c in chunks:
                    xt = px.tile([128, WP], F32R, tag="xt", name=f"xt_{m}_{c}")
                    nc.sync.dma_start(out=xt, in_=x_d[c, m])
                    xtiles[(m, c)] = xt

            def conv1_pair(k):
                ps = pp1.tile([128, W], F32, tag="ps1", name=f"ps1_{k}")
                nmm = KC * 3 * 4
                i = 0
                for c in range(KC):
                    for kx in range(3):
                        for o in range(4):
                            xt = xtiles[(2 * k + o, c)]
                            lhs = w1_sb[:, ((c * 3 + kx) * 4 + o) * 128:
                                        ((c * 3 + kx) * 4 + o + 1) * 128]
                            nc.tensor.matmul(ps, lhs, xt[:, kx:kx + W],
                                             start=(i == 0), stop=(i == nmm - 1))
                            i += 1
                sh = psh.tile([128, WP], F32R, tag="sh", name=f"sh_{k}")
                nc.vector.memset(sh[:, 0:1].bitcast(mybir.dt.uint32), 0)
                nc.vector.memset(sh[:, WP - 1:WP].bitcast(mybir.dt.uint32), 0)
                for jj in range(2):
                    j = 2 * k + jj
                    sl = slice(64 * jj, 64 * jj + HC)
                    if j == 0:           # top halo row: masked per core
                        bias, scale = cb_sb[sl, 1:2], cb_sb[sl, 3:4]
                    elif j == NSH - 1:   # bottom halo row: masked per core
                        bias, scale = cb_sb[sl, 2:3], cb_sb[sl, 4:5]
                    else:
                        bias, scale = cb_sb[sl, 0:1], 1.0
                    nc.scalar.activation(sh[sl, 1:1 + W], ps[sl, :], RELU,
                                         bias=bias, scale=scale)
                shtiles[k] = sh

            def conv23_quad(q0, nq=2):
                # outputs y = 2q0 .. 2q0+3 (two row pairs q0, q0+1)
                qs = [q0 + i for i in range(nq)]
                ps = {}
                for q in qs:
                    for yi in range(2):
                        ps[2 * q + yi] = pp23.tile([128, W], F32, tag="ps23",
                                                   name=f"ps23_{q}_{yi}")
                # all K=128 paired-tap matmuls (A) first
                for q in qs:
                    for kx in range(3):
                        for yi in range(2):
                            t = q + yi
                            lhs = w23p_sb[:, (yi * 3 + kx) * 128:
                                          (yi * 3 + kx + 1) * 128]
                            nc.tensor.matmul(ps[2 * q + yi], lhs,
                                             shtiles[t][:, kx:kx + W],
                                             start=(kx == 0), stop=False)
                # then all K=64 leftover taps as row-group pairs (B)
                for q in qs:
                    for kx in range(3):
                        for yi in range(2):
                            t = q + 1 - yi
                            p = yi
                            tap = (1 - yi) * 2 * 3 + kx
                            lhs = w23_sb[64 * p:64 * p + 64,
                                         tap * 128:(tap + 1) * 128]
                            rhs = shtiles[t][64 * p:64 * p + 64, kx:kx + W]
                            nc.tensor.matmul(ps[2 * q + yi], lhs, rhs,
                                             start=False, stop=(kx == 2))
                hm = {}
                for q in qs:
                    for yi in range(2):
                        h = phm.tile([128, W], F32R, tag="hm",
                                     name=f"hm_{q}_{yi}")
                        nc.scalar.activation(h, ps[2 * q + yi], RELU,
                                             bias=cb_sb[:, 5:6])
                        hm[2 * q + yi] = h
                return hm

            def heads_quad(q0, hm, nq=2):
                nonlocal ob
                for y in range(2 * q0, 2 * q0 + 2 * nq):
                    if ob is None:
                        ob = pob.tile([NOUT, G, W], F32, tag="ob",
                                      name=f"ob_{y}")
                    ph = pp23.tile([128, W], F32, tag="ps23", name=f"psh_{y}")
                    nc.tensor.matmul(ph[0:NOUT, :], wh_sb[:, 0:NOUT],
                                     hm[y][:, 0:W], start=True, stop=True)
                    nc.vector.tensor_scalar_add(
                        ob[:, y % G, :], ph[0:NOUT, :], cb_sb[0:NOUT, 6:7])
                    if y % G == G - 1:
                        y0 = y - G + 1
                        nc.sync.dma_start(
                            out=out_d[y0:y0 + G].rearrange("g c w -> c g w"),
                            in_=ob)
                        ob = None

            # startup order matters: the first 12 matmuls need only the
            # chunk-0 x rows and the chunk-0 half of w1
            w1_sb = pw.tile([128, 24 * 128], F32R)
            for m in range(4):
                load_x(m, chunks=(0,))
            nc.sync.dma_start(out=w1_sb[:, 0:12 * 128], in_=w1_d[:, 0:12 * 128])
            for m in range(4):
                load_x(m, chunks=(1,))
            nc.sync.dma_start(out=w1_sb[:, 12 * 128:], in_=w1_d[:, 12 * 128:])
            cb_sb = pw.tile([128, 8], F32)
            nc.sync.dma_start(out=cb_sb, in_=cb_d[:, :])
            w23p_sb = pw.tile([128, 6 * 128], F32R)
            nc.sync.dma_start(out=w23p_sb, in_=w23p_d[:, :])
            w23_sb = pw.tile([128, 9 * 128], F32R)
            nc.sync.dma_start(out=w23_sb, in_=w23_d[:, :])
            wh_sb = pw.tile([128, 16], F32R)
            nc.sync.dma_start(out=wh_sb, in_=wh_d[:, :])
            n_q = n_pairs - 1          # output row pairs
            NQ0_MAX = ((n_q - 1) // 2) * 2  # last quad start (may be 1-pair)
            NQ_LAST = n_q - NQ0_MAX
            for k in range(NQ0_MAX + 4):
                if k < n_pairs:
                    if k > 0:
                        load_x(2 * k + 2), load_x(2 * k + 3)
                    conv1_pair(k)
                    # drop input tiles no longer needed
                    for c in range(KC):
                        xtiles.pop((2 * k - 2, c), None)
                        xtiles.pop((2 * k - 1, c), None)
                if k >= 3 and k % 2 == 1 and k - 3 <= NQ0_MAX:
                    q0 = k - 3
                    nq = 2 if q0 < NQ0_MAX else NQ_LAST
                    hm = conv23_quad(q0, nq)
                    heads_quad(q0, hm, nq)
                    shtiles.pop(q0 - 1, None)

    nc.compile()
    return nc


def _prep_shards(inputs):
    f = np.float32
    bev = np.ascontiguousarray(inputs["bev_features"], dtype=f)

    def bnfold(w, g, b, m, v):
        s = (g / np.sqrt(v + EPS)).astype(f)
        return (w * s[:, None, None, None]).astype(f), (b - m * s).astype(f)

    w1f, b1 = bnfold(inputs["w_shared"], inputs["g_shared"], inputs["b_shared"],
                     inputs["m_shared"], inputs["v_shared"])
    whm, bhm = bnfold(inputs["w_hm1"], inputs["g_hm1"], inputs["b_hm1"],
                      inputs["m_hm1"], inputs["v_hm1"])
    wrg, brg = bnfold(inputs["w_reg1"], inputs["g_reg1"], inputs["b_reg1"],
                      inputs["m_reg1"], inputs["v_reg1"])

    # conv1 row-pair weights: [128 k, ((c*3+kx)*4 + o) * 128 + m]
    # column block m<64 = tap ky=o for the even row (zero when o==3),
    # block m>=64 = tap ky=o-1 for the odd row (zero when o==0)
    w1_np = np.zeros((128, 24, 128), f)
    for c in range(KC):
        for kx in range(3):
            for o in range(4):
                col = (c * 3 + kx) * 4 + o
                if o <= 2:
                    w1_np[:, col, :HC] = w1f[:, c * 128:(c + 1) * 128, o, kx].T
                if o >= 1:
                    w1_np[:, col, HC:] = w1f[:, c * 128:(c + 1) * 128, o - 1, kx].T
    w1_np = np.ascontiguousarray(w1_np.reshape(128, 24 * 128))

    # conv2+3 merged per-tap weights [64 k, tap, m] (m<64 hm1 | m>=64 reg1)
    w23_half = np.zeros((64, 9, 128), f)
    for t in range(9):
        ky, kx = divmod(t, 3)
        w23_half[:, t, :HC] = whm[:, :, ky, kx].T
        w23_half[:, t, HC:] = wrg[:, :, ky, kx].T
    # duplicated across partition halves (for K=64 matmuls at src base 0/64)
    w23_np = np.ascontiguousarray(
        np.tile(w23_half, (2, 1, 1)).reshape(128, 9 * 128))
    # K-packed vertical tap pairs: [p, par*3+kx, m] with p<64 -> tap ky=par,
    # p>=64 -> tap ky=par+1 (par = output-row parity)
    w23p_np = np.zeros((128, 6, 128), f)
    for par in range(2):
        for kx in range(3):
            w23p_np[:64, par * 3 + kx] = w23_half[:, par * 3 + kx]
            w23p_np[64:, par * 3 + kx] = w23_half[:, (par + 1) * 3 + kx]
    w23p_np = np.ascontiguousarray(w23p_np.reshape(128, 6 * 128))

    # head weights, block diagonal: [128 k, 11]
    wh_np = np.zeros((128, 16), f)
    wh_np[:HC, 0:3] = inputs["w_hm2"][:, :, 0, 0].T
    wh_np[HC:, 3:NOUT] = inputs["w_reg2"][:, :, 0, 0].T

    bh_np = np.concatenate([inputs["b_hm2"], inputs["b_reg2"]]).astype(f)

    in_maps = []
    for core in range(8):
        b, half = divmod(core, 2)
        r0 = half * HALF - 2
        x_np = np.zeros((KC, NIN, 128, WP), f)
        lo, hi = max(0, r0), min(H, r0 + NIN)
        src = bev[b, :, lo:hi, :].reshape(KC, 128, hi - lo, W)
        x_np[:, lo - r0:hi - r0, :, 1:1 + W] = src.transpose(0, 2, 1, 3)

        m_top = f(0.0 if half == 0 else 1.0)
        m_bot = f(0.0 if half == 1 else 1.0)
        cb_np = np.zeros((128, 8), f)
        b1d = np.tile(b1, 2)
        cb_np[:, 0] = b1d
        cb_np[:, 1] = b1d * m_top
        cb_np[:, 2] = b1d * m_bot
        cb_np[:, 3] = m_top
        cb_np[:, 4] = m_bot
        cb_np[:, 5] = np.concatenate([bhm, brg])
        cb_np[0:NOUT, 6] = bh_np
        cb_np[32:32 + NOUT, 6] = bh_np

        in_maps.append({"x": x_np, "w1": w1_np, "w23": w23_np,
                        "w23p": w23p_np, "wh": wh_np, "cb": cb_np})
    return in_maps


def _run(inputs, trace=False):
    global _CACHED_NC
    if _CACHED_NC is None:
        _CACHED_NC = _build_nc()
    nc = _CACHED_NC
    in_maps = _prep_shards(inputs)
    kw = {}
    if trace:
        # dev-only profiling path; requires the sibling ntff_shim module
        import os
        import shutil
        import ntff_shim  # noqa: F401
        shutil.rmtree("/tmp/ntff_kernel", ignore_errors=True)
        os.makedirs("/tmp/ntff_kernel", exist_ok=True)
        kw = dict(tmpdir="/tmp/ntff_kernel", trace_kwargs={})
    res = run_bass_kernel_spmd(nc, in_maps, list(range(8)), trace=trace, **kw)

    heatmap = np.empty((B, 3, H, W), np.float32)
    box_reg = np.empty((B, 8, H, W), np.float32)
    for core in range(8):
        b, half = divmod(core, 2)
        o = res.results[core]["out"]  # [150, 11, 300]
        rows = slice(half * HALF, half * HALF + HALF)
        heatmap[b, :, rows, :] = o[:, 0:3, :].transpose(1, 0, 2)
        box_reg[b, :, rows, :] = o[:, 3:NOUT, :].transpose(1, 0, 2)
    return (heatmap, box_reg), res


def kernel(**inputs):
    out, _ = _run(inputs)
    return out


# revision 19
# speedup vs baseline: 1.0296x; 1.0258x over previous
"""CenterPointHead Bass/Trainium2 kernel.

Reference computation (all convs stride 1):
  shared  = relu(bn(conv3x3(bev[4,256,300,300], w_shared[64,256,3,3])))
  hm      = relu(bn(conv3x3(shared, w_hm1[64,64,3,3])))
  heatmap = conv1x1(hm, w_hm2[3,64]) + b_hm2
  rg      = relu(bn(conv3x3(shared, w_reg1[64,64,3,3])))
  box_reg = conv1x1(rg, w_reg2[8,64]) + b_reg2
  returns (heatmap[4,3,300,300], box_reg[4,8,300,300])

Sharding: 8 cores = batch(4) x H-halves(2). Each core computes 150 output
rows of one image, with input halo rows (zero-filled outside the image)
so the single SPMD program is uniform across cores.

Per-core dataflow (all BN scales folded into conv weights host-side):
  conv1: shared rows computed in pairs as M=128 fp32r matmuls; lhsT columns
         0:64 carry the even row's tap, 64:128 the odd row's tap shifted by
         one input row (zero blocks at the 4-row chain boundaries).
         KC(2) x kx(3) x 4 input rows = 24 matmuls (N=300) per row pair.
  conv2+conv3 merged: output channels of hm1|reg1 stacked -> M=128.
         Shared rows stored pairwise in [128, WP] tiles (row 2t at
         partitions 0:64, row 2t+1 at 64:128) so one K=128 matmul covers two
         vertical taps; the leftover tap is a K=64 matmul at src partition
         base 0/64. 6 matmuls per output row.
  heads: w_hm2|w_reg2 block-diagonal -> one K=128, M=11 matmul per row.
"""

import numpy as np

import concourse.bacc as bacc
import concourse.mybir as mybir
import concourse.tile as tile
from concourse.bass_utils import run_bass_kernel_spmd

F32 = mybir.dt.float32
F32R = mybir.dt.float32r
RELU = mybir.ActivationFunctionType.Relu

B, C, H, W = 4, 256, 300, 300
HC = 64          # hidden channels
KC = 2           # input channel chunks of 128
NOUT = 11        # 3 heatmap + 8 box_reg channels
HALF = 150       # output rows per core
NSH = HALF + 2   # shared rows computed per core (1 halo row each side)
NIN = HALF + 4   # input rows per core (2 halo rows each side)
WP = W + 2       # width padded with one zero column each side
G = 10           # output rows batched per store DMA
EPS = 1e-5

_CACHED_NC = None


def _build_nc(n_pairs=NSH // 2):
    nc = bacc.Bacc()

    x_d = nc.dram_tensor("x", [KC, NIN, 128, WP], F32R, kind="ExternalInput")
    w1_d = nc.dram_tensor("w1", [128, 24 * 128], F32R, kind="ExternalInput")
    w23p_d = nc.dram_tensor("w23p", [128, 6 * 128], F32R, kind="ExternalInput")
    w23_d = nc.dram_tensor("w23", [128, 9 * 128], F32R, kind="ExternalInput")
    wh_d = nc.dram_tensor("wh", [128, 16], F32R, kind="ExternalInput")
    cb_d = nc.dram_tensor("cb", [128, 8], F32, kind="ExternalInput")
    out_d = nc.dram_tensor("out", [HALF, NOUT, W], F32, kind="ExternalOutput")

    with tile.TileContext(nc) as tc:
        with (
            tc.tile_pool(name="pw", bufs=1) as pw,
            tc.tile_pool(name="px", bufs=16) as px,
            tc.tile_pool(name="psh", bufs=7) as psh,
            tc.tile_pool(name="phm", bufs=8) as phm,
            tc.tile_pool(name="pob", bufs=3) as pob,
            tc.tile_pool(name="pp1", bufs=4, space="PSUM") as pp1,
            tc.tile_pool(name="pp23", bufs=4, space="PSUM") as pp23,
        ):
            xtiles = {}    # (input_row, chunk) -> AP
            shtiles = {}   # pair index -> AP ([128, WP]; row 2k at [0:64], 2k+1 at [64:128])
            ob = None

            def load_x(m, chunks=range(KC)):
                for c in chunks:
                    xt = px.tile([128, WP], F32R, tag="xt", name=f"xt_{m}_{c}")
                    nc.sync.dma_start(out=xt, in_=x_d[c, m])
                    xtiles[(m, c)] = xt

            def conv1_pair(k):
                ps = pp1.tile([128, W], F32, tag="ps1", name=f"ps1_{k}")
                nmm = KC * 3 * 4
                i = 0
                for c in range(KC):
                    for kx in range(3):
                        for o in range(4):
                            xt = xtiles[(2 * k + o, c)]
                            lhs = w1_sb[:, ((c * 3 + kx) * 4 + o) * 128:
                                        ((c * 3 + kx) * 4 + o + 1) * 128]
                            nc.tensor.matmul(ps, lhs, xt[:, kx:kx + W],
                                             start=(i == 0), stop=(i == nmm - 1))
                            i += 1
                sh = psh.tile([128, WP], F32R, tag="sh", name=f"sh_{k}")
                nc.vector.memset(sh[:, 0:1].bitcast(mybir.dt.uint32), 0)
                nc.vector.memset(sh[:, WP - 1:WP].bitcast(mybir.dt.uint32), 0)
                for jj in range(2):
                    j = 2 * k + jj
                    sl = slice(64 * jj, 64 * jj + HC)
                    if j == 0:           # top halo row: masked per core
                        bias, scale = cb_sb[sl, 1:2], cb_sb[sl, 3:4]
                    elif j == NSH - 1:   # bottom halo row: masked per core
                        bias, scale = cb_sb[sl, 2:3], cb_sb[sl, 4:5]
                    else:
                        bias, scale = cb_sb[sl, 0:1], 1.0
                    nc.scalar.activation(sh[sl, 1:1 + W], ps[sl, :], RELU,
                                         bias=bias, scale=scale)
                shtiles[k] = sh

            def conv23_quad(q0, nq=2):
                # outputs y = 2q0 .. 2q0+3 (two row pairs q0, q0+1)
                qs = [q0 + i for i in range(nq)]
                ps = {}
                for q in qs:
                    for yi in range(2):
                        ps[2 * q + yi] = pp23.tile([128, W], F32, tag="ps23",
                                                   name=f"ps23_{q}_{yi}")
                # all K=128 paired-tap matmuls (A) first
                for q in qs:
                    for kx in range(3):
                        for yi in range(2):
                            t = q + yi
                            lhs = w23p_sb[:, (yi * 3 + kx) * 128:
                                          (yi * 3 + kx + 1) * 128]
                            nc.tensor.matmul(ps[2 * q + yi], lhs,
                                             shtiles[t][:, kx:kx + W],
                                             start=(kx == 0), stop=False)
                # then all K=64 leftover taps as row-group pairs (B)
                for q in qs:
                    for kx in range(3):
                        for yi in range(2):
                            t = q + 1 - yi
                            p = yi
                            tap = (1 - yi) * 2 * 3 + kx
                            lhs = w23_sb[64 * p:64 * p + 64,
                                         tap * 128:(tap + 1) * 128]
                            rhs = shtiles[t][64 * p:64 * p + 64, kx:kx + W]
                            nc.tensor.matmul(ps[2 * q + yi], lhs, rhs,
                                             start=False, stop=(kx == 2))
                hm = {}
                for q in qs:
                    for yi in range(2):
                        h = phm.tile([128, W], F32R, tag="hm",
                                     name=f"hm_{q}_{yi}")
                        nc.scalar.activation(h, ps[2 * q + yi], RELU,
                                             bias=cb_sb[:, 5:6])
                        hm[2 * q + yi] = h
                return hm

            def heads_quad(q0, hm, nq=2):
                nonlocal ob
                for y in range(2 * q0, 2 * q0 + 2 * nq):
                    if ob is None:
                        ob = pob.tile([NOUT, G, W], F32, tag="ob",
                                      name=f"ob_{y}")
                    ph = pp23.tile([128, W], F32, tag="ps23", name=f"psh_{y}")
                    nc.tensor.matmul(ph[0:NOUT, :], wh_sb[:, 0:NOUT],
                                     hm[y][:, 0:W], start=True, stop=True)
                    nc.vector.tensor_scalar_add(
                        ob[:, y % G, :], ph[0:NOUT, :], cb_sb[0:NOUT, 6:7])
                    if y % G == G - 1:
                        y0 = y - G + 1
                        nc.sync.dma_start(
                            out=out_d[y0:y0 + G].rearrange("g c w -> c g w"),
                            in_=ob)
                        ob = None

            # startup order matters: the first 12 matmuls need only the
            # chunk-0 x rows and the chunk-0 half of w1
            w1_sb = pw.tile([128, 24 * 128], F32R)
            for m in range(4):
                load_x(m, chunks=(0,))
            nc.sync.dma_start(out=w1_sb[:, 0:12 * 128], in_=w1_d[:, 0:12 * 128])
            for m in range(4):
                load_x(m, chunks=(1,))
            nc.sync.dma_start(out=w1_sb[:, 12 * 128:], in_=w1_d[:, 12 * 128:])
            cb_sb = pw.tile([128, 8], F32)
            nc.sync.dma_start(out=cb_sb, in_=cb_d[:, :])
            w23p_sb = pw.tile([128, 6 * 128], F32R)
            nc.sync.dma_start(out=w23p_sb, in_=w23p_d[:, :])
            w23_sb = pw.tile([128, 9 * 128], F32R)
            nc.sync.dma_start(out=w23_sb, in_=w23_d[:, :])
            wh_sb = pw.tile([128, 16], F32R)
            nc.sync.dma_start(out=wh_sb, in_=wh_d[:, :])
            pending_heads = None
            n_q = n_pairs - 1          # output row pairs
            NQ0_MAX = ((n_q - 1) // 2) * 2  # last quad start (may be 1-pair)
            NQ_LAST = n_q - NQ0_MAX
            for k in range(NQ0_MAX + 4):
                if k < n_pairs:
                    if k > 0:
                        load_x(2 * k + 2), load_x(2 * k + 3)
                    conv1_pair(k)
                    # drop input tiles no longer needed
                    for c in range(KC):
                        xtiles.pop((2 * k - 2, c), None)
                        xtiles.pop((2 * k - 1, c), None)
                if pending_heads is not None:
                    heads_quad(*pending_heads)
                    pending_heads = None
                if k >= 3 and k % 2 == 1 and k - 3 <= NQ0_MAX:
                    q0 = k - 3
                    nq = 2 if q0 < NQ0_MAX else NQ_LAST
                    hm = conv23_quad(q0, nq)
                    # defer heads one iteration: the next conv1 block hides
                    # the hm-ACT latency that would otherwise stall the PE
                    pending_heads = (q0, hm, nq)
                    shtiles.pop(q0 - 1, None)
            if pending_heads is not None:
                heads_quad(*pending_heads)

    nc.compile()
    return nc


def _prep_shards(inputs):
    f = np.float32
    bev = np.ascontiguousarray(inputs["bev_features"], dtype=f)

    def bnfold(w, g, b, m, v):
        s = (g / np.sqrt(v + EPS)).astype(f)
        return (w * s[:, None, None, None]).astype(f), (b - m * s).astype(f)

    w1f, b1 = bnfold(inputs["w_shared"], inputs["g_shared"], inputs["b_shared"],
                     inputs["m_shared"], inputs["v_shared"])
    whm, bhm = bnfold(inputs["w_hm1"], inputs["g_hm1"], inputs["b_hm1"],
                      inputs["m_hm1"], inputs["v_hm1"])
    wrg, brg = bnfold(inputs["w_reg1"], inputs["g_reg1"], inputs["b_reg1"],
                      inputs["m_reg1"], inputs["v_reg1"])

    # conv1 row-pair weights: [128 k, ((c*3+kx)*4 + o) * 128 + m]
    # column block m<64 = tap ky=o for the even row (zero when o==3),
    # block m>=64 = tap ky=o-1 for the odd row (zero when o==0)
    w1_np = np.zeros((128, 24, 128), f)
    for c in range(KC):
        for kx in range(3):
            for o in range(4):
                col = (c * 3 + kx) * 4 + o
                if o <= 2:
                    w1_np[:, col, :HC] = w1f[:, c * 128:(c + 1) * 128, o, kx].T
                if o >= 1:
                    w1_np[:, col, HC:] = w1f[:, c * 128:(c + 1) * 128, o - 1, kx].T
    w1_np = np.ascontiguousarray(w1_np.reshape(128, 24 * 128))

    # conv2+3 merged per-tap weights [64 k, tap, m] (m<64 hm1 | m>=64 reg1)
    w23_half = np.zeros((64, 9, 128), f)
    for t in range(9):
        ky, kx = divmod(t, 3)
        w23_half[:, t, :HC] = whm[:, :, ky, kx].T
        w23_half[:, t, HC:] = wrg[:, :, ky, kx].T
    # duplicated across partition halves (for K=64 matmuls at src base 0/64)
    w23_np = np.ascontiguousarray(
        np.tile(w23_half, (2, 1, 1)).reshape(128, 9 * 128))
    # K-packed vertical tap pairs: [p, par*3+kx, m] with p<64 -> tap ky=par,
    # p>=64 -> tap ky=par+1 (par = output-row parity)
    w23p_np = np.zeros((128, 6, 128), f)
    for par in range(2):
        for kx in range(3):
            w23p_np[:64, par * 3 + kx] = w23_half[:, par * 3 + kx]
            w23p_np[64:, par * 3 + kx] = w23_half[:, (par + 1) * 3 + kx]
    w23p_np = np.ascontiguousarray(w23p_np.reshape(128, 6 * 128))

    # head weights, block diagonal: [128 k, 11]
    wh_np = np.zeros((128, 16), f)
    wh_np[:HC, 0:3] = inputs["w_hm2"][:, :, 0, 0].T
    wh_np[HC:, 3:NOUT] = inputs["w_reg2"][:, :, 0, 0].T

    bh_np = np.concatenate([inputs["b_hm2"], inputs["b_reg2"]]).astype(f)

    in_maps = []
    for core in range(8):
        b, half = divmod(core, 2)
        r0 = half * HALF - 2
        x_np = np.zeros((KC, NIN, 128, WP), f)
        lo, hi = max(0, r0), min(H, r0 + NIN)
        src = bev[b, :, lo:hi, :].reshape(KC, 128, hi - lo, W)
        x_np[:, lo - r0:hi - r0, :, 1:1 + W] = src.transpose(0, 2, 1, 3)

        m_top = f(0.0 if half == 0 else 1.0)
        m_bot = f(0.0 if half == 1 else 1.0)
        cb_np = np.zeros((128, 8), f)
        b1d = np.tile(b1, 2)
        cb_np[:, 0] = b1d
        cb_np[:, 1] = b1d * m_top
        cb_np[:, 2] = b1d * m_bot
        cb_np[:, 3] = m_top
        cb_np[:, 4] = m_bot
        cb_np[:, 5] = np.concatenate([bhm, brg])
        cb_np[0:NOUT, 6] = bh_np
        cb_np[32:32 + NOUT, 6] = bh_np

        in_maps.append({"x": x_np, "w1": w1_np, "w23": w23_np,
                        "w23p": w23p_np, "wh": wh_np, "cb": cb_np})
    return in_maps


def _run(inputs, trace=False):
    global _CACHED_NC
    if _CACHED_NC is None:
        _CACHED_NC = _build_nc()
    nc = _CACHED_NC
    in_maps = _prep_shards(inputs)
    kw = {}
    if trace:
        # dev-only profiling path; requires the sibling ntff_shim module
        import os
        import shutil
        import ntff_shim  # noqa: F401
        shutil.rmtree("/tmp/ntff_kernel", ignore_errors=True)
        os.makedirs("/tmp/ntff_kernel", exist_ok=True)
        kw = dict(tmpdir="/tmp/ntff_kernel", trace_kwargs={})
    res = run_bass_kernel_spmd(nc, in_maps, list(range(8)), trace=trace, **kw)

    heatmap = np.empty((B, 3, H, W), np.float32)
    box_reg = np.empty((B, 8, H, W), np.float32)
    for core in range(8):
        b, half = divmod(core, 2)
        o = res.results[core]["out"]  # [150, 11, 300]
        rows = slice(half * HALF, half * HALF + HALF)
        heatmap[b, :, rows, :] = o[:, 0:3, :].transpose(1, 0, 2)
        box_reg[b, :, rows, :] = o[:, 3:NOUT, :].transpose(1, 0, 2)
    return (heatmap, box_reg), res


def kernel(**inputs):
    out, _ = _run(inputs)
    return out
